# revision 1
# baseline (speedup 1.0000x reference)
"""Trainium2 Bass kernel: batched bidirectional cross-attention (sparse_attention).

Math per batch b (N=90 nodes, D=32 feat):
  S = sc[b]            [N, D]
  F = fc[b]            [N, D]
  H = F @ W_e          [N, D]            (so A = S @ W_e^T @ F^T = S @ H^T)
  A = S @ H^T          [N, N]
  cosc[b] = softmax_col(A)^T @ S  = diag(1/c) E^T S,  E = exp(A), c = colsum(E)
  cofc[b] = softmax_row(A)  @ F   = diag(1/r) Ê^T F,  Ê = exp(A^T), r = rowsum(E)

Mapping: fp16 logit path (PE 1 cyc/col), bf16 exponentials (need fp32-like
range: |A| can reach ~40 so E spans e^±40), fp32 PSUM everywhere, fp32 I/O.
Softmax max-subtraction is skipped (exact in exact arithmetic; bf16/fp32
ranges absorb e^40).

Per 4-batch "slab": one [90,128]->[128,90] PE transpose per input puts 4
batches' S^T/F^T at the 4 partition strips (32*tau), feeding tile_position
row-group-packed K=32 matmuls for A and A^T. Stage-2 uses E/Ê as stationary
with a ones-augmented rhs [S|1] so softmax sums fall out of the matmul;
normalization is one strided DVE tensor_tensor with a step-0 broadcast AP.
"""

import numpy as np

import concourse.bass as bass
import concourse.mybir as mybir
import concourse.tile as tile
from concourse import bacc
from concourse.masks import make_identity

NUM_NODE = 90
FEAT_DIM = 32
BATCH = 8192
N_CORES = 8
B_CORE = BATCH // N_CORES  # 1024 batches per core
MG = 64                    # batches per DMA megagroup

FP32 = mybir.dt.float32
FP16 = mybir.dt.float16
BF16 = mybir.dt.bfloat16

N = NUM_NODE   # 90
D = FEAT_DIM   # 32


def build_kernel(b_core: int = B_CORE, mg: int = MG, debug_dump: bool = False,
                 repeat: int = 1, hw_loop: bool = False):
    """Build the single-core Bass module processing b_core batches.

    repeat > 1 re-runs the whole computation that many times inside the NEFF
    (same data, same outputs) — benchmarking only, to amortize launch cost.
    hw_loop uses a For_i hardware loop for the repeats.
    """
    assert mg % 16 == 0 and b_core % mg == 0
    nc = bacc.Bacc(None, target_bir_lowering=False)
    dbg = {}
    if debug_dump:
        dbg["st"] = nc.dram_tensor("dbg_st", [128, 360], FP32, kind="ExternalOutput")
        dbg["ht"] = nc.dram_tensor("dbg_ht", [128, 360], FP32, kind="ExternalOutput")
        dbg["a"] = nc.dram_tensor("dbg_a", [N, 360], FP32, kind="ExternalOutput")
        dbg["at"] = nc.dram_tensor("dbg_at", [N, 360], FP32, kind="ExternalOutput")
        dbg["e"] = nc.dram_tensor("dbg_e", [N, 360], FP32, kind="ExternalOutput")
        dbg["u"] = nc.dram_tensor("dbg_u", [N, 264], FP32, kind="ExternalOutput")

    sc = nc.dram_tensor("sc", [b_core * N, D], FP32, kind="ExternalInput")
    fc = nc.dram_tensor("fc", [b_core * N, D], FP32, kind="ExternalInput")
    w = nc.dram_tensor("w", [D, D], FP32, kind="ExternalInput")
    cosc = nc.dram_tensor("cosc", [b_core * N, D], FP32, kind="ExternalOutput")
    cofc = nc.dram_tensor("cofc", [b_core * N, D], FP32, kind="ExternalOutput")

    # n-major views: [n, b, d]
    sc_v = sc[:, :].rearrange("(b n) d -> n b d", n=N)
    fc_v = fc[:, :].rearrange("(b n) d -> n b d", n=N)
    cosc_v = cosc[:, :].rearrange("(b n) d -> n b d", n=N)
    cofc_v = cofc[:, :].rearrange("(b n) d -> n b d", n=N)

    nmg = b_core // mg
    Exp = mybir.ActivationFunctionType.Exp

    with tile.TileContext(nc) as tc:
        with (
            tc.tile_pool(name="singles", bufs=1) as singles,
            tc.tile_pool(name="io", bufs=2) as io,
            tc.tile_pool(name="c16", bufs=2) as c16p,
            tc.tile_pool(name="rhsp", bufs=2) as rhsp,
            tc.tile_pool(name="tsb", bufs=2) as tsbp,
            tc.tile_pool(name="ep", bufs=2) as ep,
            tc.tile_pool(name="rp", bufs=2) as rp,
            tc.tile_pool(name="tp", bufs=2, space="PSUM") as tpp,
            tc.tile_pool(name="hp", bufs=1, space="PSUM") as hpp,
            tc.tile_pool(name="ap", bufs=3, space="PSUM") as app,
            tc.tile_pool(name="up", bufs=2, space="PSUM") as upp,
        ):
            # ---- one-time constants ----
            ident16 = singles.tile([128, 128], FP16)
            make_identity(nc, ident16)
            ident_bf = singles.tile([128, 128], BF16)
            make_identity(nc, ident_bf)

            # W^T (= [d_in, d_out]) as fp16, replicated at the 4 partition
            # strips for the diagonal tile_position transform matmuls.
            # lhsT for the transform: out = lhsT.T @ rhs needs lhsT[e, d] =
            # W_e[e, d], i.e. W_e in natural layout.
            wt16 = singles.tile([128, D], FP16)
            for t in range(4):
                nc.gpsimd.dma_start(out=wt16[32 * t:32 * t + 32, :], in_=w[:, :])

            zbias = singles.tile([128, 1], FP32)
            nc.vector.memset(zbias, 0.0)

            def do_mg(m):
                # ---- load + cast ----
                s_nat = io.tile([N, mg * D], FP32, tag="nat")
                f_nat = io.tile([N, mg * D], FP32, tag="nat")
                nc.sync.dma_start(
                    out=s_nat.rearrange("n (b d) -> n b d", d=D),
                    in_=sc_v[:, m * mg:(m + 1) * mg, :],
                )
                nc.sync.dma_start(
                    out=f_nat.rearrange("n (b d) -> n b d", d=D),
                    in_=fc_v[:, m * mg:(m + 1) * mg, :],
                )

                s16 = c16p.tile([N, mg * D], FP16, tag="c16")
                f16 = c16p.tile([N, mg * D], FP16, tag="c16")
                nc.gpsimd.tensor_copy(s16, s_nat)
                nc.gpsimd.tensor_copy(f16, f_nat)

                # stage-2 rhs [S|1], [F|1] in bf16: [n, (b, 33)]
                sb1 = rhsp.tile([N, mg * (D + 1)], BF16, tag="rhs")
                fb1 = rhsp.tile([N, mg * (D + 1)], BF16, tag="rhs")
                sb1_v = sb1.rearrange("n (b d) -> n b d", d=D + 1)
                fb1_v = fb1.rearrange("n (b d) -> n b d", d=D + 1)
                nc.gpsimd.tensor_copy(
                    sb1_v[:, :, 0:D], s_nat.rearrange("n (b d) -> n b d", d=D)
                )
                nc.gpsimd.tensor_copy(
                    fb1_v[:, :, 0:D], f_nat.rearrange("n (b d) -> n b d", d=D)
                )
                nc.vector.memset(sb1_v[:, :, D:D + 1], 1.0)
                nc.vector.memset(fb1_v[:, :, D:D + 1], 1.0)

                stage = io.tile([N, 2 * mg * D], FP32, tag="stage")

                for g in range(mg // 16):
                    # ---- transposes: 4 slabs x [90,128] -> [128,90] ----
                    ts = tpp.tile([128, 360], FP16, tag="tp")
                    tf = tpp.tile([128, 360], FP16, tag="tp")
                    for k in range(4):
                        c0 = (g * 16 + 4 * k) * D
                        nc.tensor.matmul(
                            ts[:, 90 * k:90 * k + 90],
                            s16[:, c0:c0 + 128],
                            ident16[0:90, 0:90],
                            is_transpose=True,
                        )
                        nc.tensor.matmul(
                            tf[:, 90 * k:90 * k + 90],
                            f16[:, c0:c0 + 128],
                            ident16[0:90, 0:90],
                            is_transpose=True,
                        )
                    st = tsbp.tile([128, 360], FP16, tag="tsb")
                    ft = tsbp.tile([128, 360], FP16, tag="tsb")
                    nc.vector.tensor_copy(st, ts)
                    nc.scalar.copy(ft, tf)

                    # ---- transform: H^T = W^T F^T, diagonal tile_position ----
                    h = hpp.tile([128, 512], FP32)
                    for t in range(4):
                        nc.tensor.matmul(
                            h[32 * t:32 * t + 32, 0:360],
                            wt16[32 * t:32 * t + 32, :],
                            ft[32 * t:32 * t + 32, :],
                            tile_position=(32 * t, 32 * t),
                        )
                    ht = tsbp.tile([128, 360], FP16, tag="tsb")
                    nc.vector.tensor_copy(ht, h[:, 0:360])

                    if debug_dump and m == 0 and g == 0:
                        for nm, src in (("st", st), ("ht", ht)):
                            dt_ = io.tile([128, 360], FP32, tag="dbg")
                            nc.vector.tensor_copy(dt_, src)
                            nc.sync.dma_start(out=dbg[nm][:, :], in_=dt_)

                    for t in range(4):
                        # ---- A and A^T for the 4 batches at strip t ----
                        # All matmuls into one PSUM bank must share a PE
                        # row-group: concurrent row-groups draining into the
                        # same partitions of a bank hard-fault the device.
                        a_ps = app.tile([N, 360], FP32, tag="a")
                        for k in range(4):
                            nc.tensor.matmul(
                                a_ps[:, 90 * k:90 * k + 90],
                                st[32 * t:32 * t + 32, 90 * k:90 * k + 90],
                                ht[32 * t:32 * t + 32, 90 * k:90 * k + 90],
                                tile_position=(32 * t, 0),
                            )
                        # ---- exponential (bf16, no max subtraction) ----
                        e_sb = ep.tile([N, 360], BF16, tag="e")
                        nc.scalar.activation(e_sb, a_ps, Exp, bias=zbias[0:N, :])

                        # ---- Ê = E^T via PE transpose (exp commutes with T;
                        # saves the A^T matmuls' ldweights + a second exp) ----
                        et_ps = app.tile([N, 360], BF16, tag="a")
                        for k in range(4):
                            nc.tensor.matmul(
                                et_ps[:, 90 * k:90 * k + 90],
                                e_sb[:, 90 * k:90 * k + 90],
                                ident_bf[0:90, 0:90],
                                is_transpose=True,
                            )
                        et_sb = ep.tile([N, 360], BF16, tag="e")
                        nc.scalar.copy(et_sb, et_ps)

                        # ---- stage 2: U = E^T [S|1], V = Ê^T [F|1] ----
                        u_ps = upp.tile([N, 264], FP32)
                        for k in range(4):
                            b = g * 16 + 4 * k + t
                            nc.tensor.matmul(
                                u_ps[:, 33 * k:33 * k + 33],
                                e_sb[:, 90 * k:90 * k + 90],
                                sb1_v[:, b, :],
                            )
                            nc.tensor.matmul(
                                u_ps[:, 132 + 33 * k:132 + 33 * k + 33],
                                et_sb[:, 90 * k:90 * k + 90],
                                fb1_v[:, b, :],
                            )

                        if debug_dump and m == 0 and g == 0 and t == 0:
                            for nm, src in (("a", a_ps), ("u", u_ps)):
                                dt_ = io.tile([N, 360], FP32, tag="dbg")
                                w_ = src.free_size()
                                nc.vector.tensor_copy(dt_[:, 0:w_], src)
                                nc.sync.dma_start(out=dbg[nm][:, :], in_=dt_[:, 0:w_])
                            dt_ = io.tile([N, 360], FP32, tag="dbg")
                            nc.vector.tensor_copy(dt_, e_sb)
                            nc.sync.dma_start(out=dbg["e"][:, :], in_=dt_)

                        # ---- normalize: out = U[:, :32] * (1 / U[:, 32]) ----
                        u_v = u_ps.rearrange("n (x t c) -> n x t c", x=2, c=33)
                        rec = rp.tile([N, 8], FP32, tag="r")
                        rec_v = rec.rearrange("n (x t) -> n x t", x=2)
                        nc.vector.reciprocal(rec_v, u_v[:, :, :, D:D + 1].rearrange(
                            "n x t c -> n x (t c)"))
                        rec_b = bass.AP(
                            tensor=rec.tensor,
                            offset=rec.offset,
                            ap=[rec.ap[0], [4, 2], [1, 4], [0, D]],
                        )
                        b0 = g * 16 + t
                        stage_out = bass.AP(
                            tensor=stage.tensor,
                            offset=stage.offset + b0 * D,
                            ap=[stage.ap[0], [mg * D, 2], [4 * D, 4], [1, D]],
                        )
                        nc.vector.tensor_mul(stage_out, u_v[:, :, :, 0:D], rec_b)

                # ---- store ----
                stage_v = stage.rearrange("n (b d) -> n b d", d=D)
                nc.sync.dma_start(
                    out=cosc_v[:, m * mg:(m + 1) * mg, :],
                    in_=stage_v[:, 0:mg, :],
                )
                nc.sync.dma_start(
                    out=cofc_v[:, m * mg:(m + 1) * mg, :],
                    in_=stage_v[:, mg:2 * mg, :],
                )

            if hw_loop and repeat > 1:
                with tc.For_i(0, repeat, 1):
                    for m in range(nmg):
                        do_mg(m)
            else:
                for m_rep in range(repeat * nmg):
                    do_mg(m_rep % nmg)

    nc.compile()
    return nc


_CACHE = {}


def kernel(sc_feats: np.ndarray, fc_feats: np.ndarray, W_e: np.ndarray):
    from concourse.bass_utils import run_bass_kernel_spmd

    if "nc" not in _CACHE:
        _CACHE["nc"] = build_kernel(B_CORE, MG)
    nc = _CACHE["nc"]

    scr = np.ascontiguousarray(
        sc_feats.reshape(N_CORES, B_CORE * N, D), dtype=np.float32
    )
    fcr = np.ascontiguousarray(
        fc_feats.reshape(N_CORES, B_CORE * N, D), dtype=np.float32
    )
    w = np.ascontiguousarray(W_e, dtype=np.float32)
    in_maps = [
        {"sc": scr[c], "fc": fcr[c], "w": w} for c in range(N_CORES)
    ]
    res = run_bass_kernel_spmd(nc, in_maps, core_ids=list(range(N_CORES)))
    cosc = np.concatenate([r["cosc"] for r in res.results], axis=0)
    cofc = np.concatenate([r["cofc"] for r in res.results], axis=0)
    return cosc, cofc

